# revision 24
# baseline (speedup 1.0000x reference)
"""Conv2D-KAN Trainium2 kernel (8-core data-parallel SPMD), v4.

Formulation (per 3x3 patch, in_size = 288):
    out[n,o] = sum_{i,k} B[n,i,k] * (spline_kernel*scale)[i,k,o]
             + silu(xf) @ scale_factor + biases
with B the cubic B-spline basis (8 funcs) over uniform knots
t_r = -2.2 + 0.4 r (r = 0..11).

Key tricks vs v3 (87.8us):
 1. x clipped to the grid range [-2.2, 2.2]: all 8 basis functions
    vanish identically at and beyond both grid edges, so
    B_k(clip(x)) == B_k(x) exactly.  This caps the truncated powers
    T_r = (x - t_r)_+^3 at 4.4^3 = 85, small enough that a SINGLE
    f32r (FP22) stream through the PE blend keeps the 4th-difference
    cancellation within tolerance (measured end-to-end 6e-3 in numpy).
    Kills v3's hi/lo double stream: half the blend matmuls, no hi
    casts (DVE), no lo subs (GpSimd).
 2. T_r = sq * vxp with sq = Square(x + bias_r) on ACT (free affine)
    and vxp = relu(x - t_r) via one dual-op tensor_scalar
    (sub then max) -- 1 ACT + 1 ts + 1 TT per r-tile.
 3. Main conv in fp8e4 DoubleRow (0.5 cyc/row): the two 4k-groups of
    the basis are the two DoubleRow k-tiles ([128, 2, N] APs).  Conv
    windows stream 480 contiguous columns per 15-row half (32-pitch
    rows, 2 garbage cols/row dropped at the psum->out copy), which
    keeps the DoubleRow rhs AP 3D.
 4. Biases folded into a ones-row of the silu chunk (K=97); psum
    carries out*G (G=512); the psum->out copy applies 1/G and emits
    bf16 (host upcasts).
 5. ~28 tiny f32 warm-up matmuls at t=0 ramp the PE HAM clock gate to
    2.4 GHz before the first real blend matmul (~5us in); v3 ran its
    first 32us at 1.2 GHz.

Output y [128, 3600] bf16 per core is reassembled on host.
"""

import sys

sys.path.insert(0, "/opt/trn_rl_repo")

import numpy as np

N_CORES = 8
B, HH, WW, C = 32, 32, 32, 32
F = 128
KH = KW = 3
HO, WO = HH - KH + 1, WW - KW + 1          # 30, 30
BPC = B // N_CORES                          # images per core = 4
PIX = HH * WW                               # 1024 pixels per image
NPC = BPC * HO * WO                         # 3600 patches per core
HGRID = 0.4
T0 = -2.2                                   # first knot
XCLIP = 2.2
GSC = 512.0                                 # global psum scale
SB = 128.0                                  # B-value scale into e4m3
NWARM = 14                                  # PE HAM warm-up matmuls

_cache = {}


def _build_program():
    import concourse.bacc as bacc
    import concourse.mybir as mybir
    import concourse.tile as tile

    f32 = mybir.dt.float32
    f32r = mybir.dt.float32r
    bf16 = mybir.dt.bfloat16
    f16 = mybir.dt.float16
    f8 = mybir.dt.float8e4
    AF = mybir.ActivationFunctionType
    ALU = mybir.AluOpType
    DR = mybir.MatmulPerfMode.DoubleRowSwInterleave

    nc = bacc.Bacc("TRN2", target_bir_lowering=False, debug=False)
    xt = nc.dram_tensor("xt", [C, BPC * PIX], f16, kind="ExternalInput").ap()
    # spline weights, tap-major: [128 = 4k x 32c, 9 taps * 2 groups * F]
    wsp = nc.dram_tensor("wsp", [128, 9 * 2 * F], f8, kind="ExternalInput").ap()
    # silu weights + bias row: [97 = 3dj x 32c + ones, 3 di * F]
    wsl = nc.dram_tensor("wsl", [97, 3 * F], f16, kind="ExternalInput").ap()
    # banded blend matrices -Ma|-Mb|Ma|Mb (cm4 = [1,-4,6,-4,1])
    wm = nc.dram_tensor("wm", [128, 4 * 128], f16, kind="ExternalInput").ap()
    consts = nc.dram_tensor("consts", [128, 8], f32, kind="ExternalInput").ap()
    ones1 = nc.dram_tensor("ones1", [1, BPC * PIX], f16,
                           kind="ExternalInput").ap()
    y = nc.dram_tensor("y", [F, NPC], bf16, kind="ExternalOutput").ap()

    with tile.TileContext(nc) as tc:
        with (
            tc.tile_pool(name="wp", bufs=1) as wp,
            tc.tile_pool(name="xcp", bufs=1) as xcp,
            tc.tile_pool(name="sqp", bufs=1) as sqp,
            tc.tile_pool(name="vpp", bufs=1) as vpp,
            tc.tile_pool(name="tp", bufs=1) as tp,
            tc.tile_pool(name="b2p", bufs=2) as b2p,
            tc.tile_pool(name="op", bufs=1) as op_,
            tc.tile_pool(name="pb", bufs=1, space="PSUM") as pb,
            tc.tile_pool(name="pc", bufs=2, space="PSUM") as pc,
        ):
            # junk tile for PE warm-up: no DMA dependency
            junk = wp.tile([128, 256], f16, tag="junk")
            nc.gpsimd.memset(junk[:], 0.5)

            ct = wp.tile([128, 8], f32)
            nc.scalar.dma_start(ct[:], consts[:])

            # warm the ACT table set (silu / square / identity / copy).
            warm = wp.tile([1, 1], f32, tag="warm")
            nc.scalar.activation(warm[:], ct[:1, :1], AF.Silu)

            # ~18 N=256 f16 matmuls (~3.8us at the cold clock) ramp the
            # HAM clock gate to 8/8 while the DMA + elementwise prologue
            # runs; ends right as the first real blend matmul arrives.
            pw = pb.tile([128, 1024], f32, tag="pb0", name="pbw")
            for w_i in range(NWARM):
                nc.tensor.matmul(pw[:, 0:256], junk[:, 0:128],
                                 junk[:, 0:256], start=True, stop=True)

            # blend matrices first: needed by the first real matmul
            wmt = wp.tile([128, 4 * 128], f16, tag="wm")
            nc.gpsimd.dma_start(wmt[:], wm[:])

            # x replicated to [128 = 4r x 32c, 4096]: 16 DMAs, image-major
            # so image 0 is resident early.
            xr4 = wp.tile([128, BPC * PIX], f16, tag="xr4")
            eng = [nc.sync, nc.gpsimd]
            wslt = wp.tile([97, 3 * F], f16, tag="wsl")
            wspt = wp.tile([128, 9 * 2 * F], f8, tag="wsp")
            q = 0
            for im_d in range(BPC):
                dsl = slice(im_d * PIX, (im_d + 1) * PIX)
                for rep in range(4):
                    eng[q % 2].dma_start(
                        xr4[32 * rep:32 * rep + 32, dsl], xt[:, dsl])
                    q += 1
                if im_d == 0:
                    nc.gpsimd.dma_start(wslt[:], wsl[:])
                    nc.gpsimd.dma_start(wspt[:], wsp[:])
            stats = [wmt[:, 128 * i:128 * (i + 1)] for i in range(4)]

            # silu tile: dj-shifted partition groups; ones bias row DMAed
            sl3 = wp.tile([97, BPC * PIX], f16, tag="sl3")
            nc.sync.dma_start(sl3[96:97, :], ones1[:])
            # pair-tail cells the shifted copies never write (only ever
            # read as dropped garbage columns, but must be initialized)
            for end_ in (1, 3, 4):
                for dj_ in (1, 2):
                    nc.vector.memset(
                        sl3[32 * dj_:32 * dj_ + 32,
                            end_ * PIX - 4:end_ * PIX], 0.0)

            out_t = op_.tile([F, NPC], bf16)

            blendT = {}
            convst = {}

            def emit_elem(im0, nim):
                # elementwise for a group of nim images
                W = nim * PIX
                sl = slice(im0 * PIX, im0 * PIX + W)
                xc = xcp.tile([128, W], f32, tag=f"xc{im0}", name="xc")
                nc.vector.tensor_scalar(
                    xc[:], xr4[:, sl], -XCLIP, XCLIP, ALU.max, ALU.min)
                # mirror-split streams: left powers -(t_r-x)_+^3 feed the
                # low-k basis group (r 0..7), right powers (x-t_r)_+^3 the
                # high-k group (r 4..11).  All values <= 22 -> f16 chain.
                sqs = []
                for t in range(3):
                    sq = sqp.tile([128, W], f16, tag=f"sq{t}_{im0}",
                                  name="sq")
                    nc.scalar.activation(
                        sq[:], xc[:], AF.Square,
                        bias=ct[:, t:t + 1], scale=1.0)
                    sqs.append(sq)
                Ts = []
                for s_i, (t, alu) in enumerate(
                        ((0, ALU.min), (1, ALU.min), (1, ALU.max),
                         (2, ALU.max))):
                    vx = vpp.tile([128, W], f16, tag=f"vx{s_i}_{im0}",
                                  name="vx")
                    nc.vector.tensor_scalar(
                        vx[:], xc[:], ct[:, t:t + 1], 0.0,
                        ALU.add, alu)
                    T = tp.tile([128, W], f16, tag=f"T{s_i}_{im0}", name="T")
                    nc.vector.tensor_mul(T[:], sqs[t][:], vx[:])
                    Ts.append(T)
                # silu into group 0, dj-shifts via cheap f16 copies
                nc.scalar.activation(
                    sl3[0:32, sl], xr4[0:32, sl], AF.Silu)
                for dj in (1, 2):
                    n = W - dj
                    nc.vector.tensor_copy(
                        sl3[32 * dj:32 * dj + 32,
                            sl.start:sl.start + n],
                        sl3[0:32, sl.start + dj:sl.start + dj + n])
                for i_ in range(nim):
                    blendT[im0 + i_] = (Ts, i_ * PIX)

            def emit_blend(im, h):
                # one 512-col chunk of image im's blend (both groups)
                Ts, off = blendT[im]
                base = off + h * 512
                hs = slice(base, base + 512)
                pbg = [pb.tile([128, 512], f32, tag=f"pb{g}",
                               name=f"pb{g}_{im}{h}") for g in range(2)]
                for g, (sa, sb_, Ta, Tb) in enumerate(
                        ((stats[0], stats[1], Ts[0], Ts[1]),
                         (stats[2], stats[3], Ts[2], Ts[3]))):
                    nc.tensor.matmul(pbg[g][:], sa, Ta[:, hs],
                                     start=True, stop=False)
                    nc.tensor.matmul(pbg[g][:], sb_, Tb[:, hs],
                                     start=False, stop=True)
                return pbg

            def emit_bcopy(im, h, pbg):
                if im not in convst:
                    convst[im] = b2p.tile([128, 2 * PIX], f8, tag="b2", name="b2")
                b2 = convst[im]
                for g in range(2):
                    nc.scalar.activation(
                        b2[:, g * PIX + h * 512:g * PIX + (h + 1) * 512],
                        pbg[g][:], AF.Identity, scale=SB)

            def emit_blend_img(im):
                for h in range(2):
                    pbg = emit_blend(im, h)
                    emit_bcopy(im, h, pbg)

            def emit_conv(im):
                b2v = convst.pop(im)[:].rearrange("p (g x) -> p g x", g=2)
                pss = [pc.tile([F, 480], f32, tag=f"ps{hf}",
                               name=f"ps{hf}") for hf in range(2)]
                for di in range(3):
                    for hf in range(2):
                        base = (hf * 15 + di) * 32
                        nc.tensor.matmul(
                            pss[hf][:], wslt[:, di * F:(di + 1) * F],
                            sl3[:, im * PIX + base:im * PIX + base + 480],
                            start=(di == 0), stop=False)
                    for dj in range(3):
                        tap = di * 3 + dj
                        lhsT = wspt[:, tap * 256:(tap + 1) * 256]
                        for hf in range(2):
                            base = (hf * 15 + di) * 32
                            # trim runs passing the image end -- only
                            # garbage cols (ww>=30) lost
                            ln = min(480, PIX - base - dj)
                            nc.tensor.matmul(
                                pss[hf][:, 0:ln], lhsT,
                                b2v[:, :, base + dj:base + dj + ln],
                                start=False, stop=(tap == 8 and hf == 1),
                                perf_mode=DR)
                return pss

            def emit_out(im, pss):
                for hf in range(2):
                    s = (im * 2 + hf) * 450
                    pv = pss[hf][:].rearrange(
                        "p (h w) -> p h w", w=32)[:, :, 0:30]
                    dst = out_t[:, s:s + 450]
                    nc.scalar.activation(
                        dst, pv, AF.Identity, scale=1.0 / GSC)
                    nc.sync.dma_start(y[:, s:s + 450], dst)

            # software pipeline: elementwise groups [0],[1,2],[3];
            # conv(im) fills the PE while the next group's chain runs
            emit_elem(0, 1)
            emit_blend_img(0)
            emit_elem(1, 2)
            ps0 = emit_conv(0)
            emit_blend_img(1)
            emit_out(0, ps0)
            ps1 = emit_conv(1)
            emit_blend_img(2)
            emit_elem(3, 1)
            emit_out(1, ps1)
            ps2 = emit_conv(2)
            emit_blend_img(3)
            emit_out(2, ps2)
            ps3 = emit_conv(3)
            emit_out(3, ps3)

    nc.compile()
    return nc


def _prep_static(spline_kernel, scale_factor, kan_bias, conv_bias):
    import ml_dtypes

    e4 = ml_dtypes.float8_e4m3
    h3 = HGRID ** 3
    wsc = GSC / (6.0 * h3 * SB)
    w6 = (spline_kernel.astype(np.float64)
          * scale_factor.astype(np.float64)[:, None, :]) * wsc
    w6 = w6.reshape(KH * KW, C, 8, F)                    # (tap, c, k, F)
    Wsp = np.zeros((128, 9, 2, F), np.float64)
    for tap in range(9):
        for g in range(2):
            blk = w6[tap][:, 4 * g:4 * g + 4]            # (32c, 4k, F)
            Wsp[:, tap, g] = blk.transpose(1, 0, 2).reshape(128, F)
    # DoubleRowSwInterleave weight layout: per column j (reverse filter
    # order), the k-tile pair [A_{127-j}, B_{127-j}] interleaved
    Wsw = np.zeros((128, 9, F, 2), np.float64)
    Wsw[:, :, :, 0] = Wsp[:, :, 0, ::-1]
    Wsw[:, :, :, 1] = Wsp[:, :, 1, ::-1]
    wsp = np.ascontiguousarray(
        np.clip(Wsw, -240, 240).reshape(128, 9 * 2 * F)).astype(e4)

    sf9 = (scale_factor.astype(np.float64) * GSC).reshape(3, 3, C, F)
    Wsl = np.zeros((97, 3, F), np.float64)
    for di in range(3):
        for dj in range(3):
            Wsl[32 * dj:32 * dj + 32, di] = sf9[di, dj]
    Wsl[96, 0] = GSC * (kan_bias.astype(np.float64)
                        + conv_bias.astype(np.float64))
    wsl = np.ascontiguousarray(
        Wsl.reshape(97, 3 * F)).astype(np.float16)

    cm = np.array([1.0, -4.0, 6.0, -4.0, 1.0])
    pin = np.arange(128)[:, None]
    pout = np.arange(128)[None, :]
    same_c = (pin % 32) == (pout % 32)
    Ms = []
    for base in (0, 4):
        m = base + pin // 32 - pout // 32
        Ms.append(np.where((m >= 0) & (m <= 4) & same_c,
                           cm[np.clip(m, 0, 4)], 0.0))
    wm = np.ascontiguousarray(
        np.concatenate([-Ms[0], -Ms[1], Ms[0], Ms[1]], axis=1)
    ).astype(np.float16)                                 # [128, 512]

    consts = np.zeros((128, 8), np.float32)
    p = np.arange(128)
    for t in range(3):
        r = 4 * t + p // 32
        consts[:, t] = -(T0 + HGRID * r)                 # x-units
    return wsp, wsl, wm, consts


def kernel(x, spline_kernel, scale_factor, kan_bias, conv_bias):
    from concourse import bass_utils

    x = np.asarray(x, np.float32)
    spline_kernel = np.asarray(spline_kernel, np.float32)
    scale_factor = np.asarray(scale_factor, np.float32)
    kan_bias = np.asarray(kan_bias, np.float32)
    conv_bias = np.asarray(conv_bias, np.float32)

    if "nc" not in _cache:
        _cache["nc"] = _build_program()
    nc = _cache["nc"]

    wsp, wsl, wm, consts = _prep_static(
        spline_kernel, scale_factor, kan_bias, conv_bias)

    in_maps = []
    for c in range(N_CORES):
        xc = x[c * BPC:(c + 1) * BPC]                     # (4,32,32,32)
        xtc = np.ascontiguousarray(
            xc.transpose(3, 0, 1, 2).reshape(C, BPC * PIX), np.float16
        )
        in_maps.append(
            {"xt": xtc, "wsp": wsp, "wsl": wsl, "wm": wm, "consts": consts,
             "ones1": np.ones((1, BPC * PIX), np.float16)})

    res = bass_utils.run_bass_kernel_spmd(
        nc, in_maps, core_ids=list(range(N_CORES)),
        **_cache.get("run_kwargs", {})
    )
    _cache["last_result"] = res

    out = np.empty((B, HO, WO, F), np.float32)
    for c in range(N_CORES):
        yc = np.asarray(res.results[c]["y"]).astype(np.float32)  # (128,3600)
        out[c * BPC:(c + 1) * BPC] = (
            yc.reshape(F, BPC, 2, 15, WO).transpose(1, 2, 3, 4, 0)
            .reshape(BPC, HO, WO, F)
        )
    return out
